# revision 9
# baseline (speedup 1.0000x reference)
import sys

if "/opt/trn_rl_repo" not in sys.path:
    sys.path.insert(0, "/opt/trn_rl_repo")

import numpy as np
import ml_dtypes

B, S, D, H = 2, 2048, 1024, 16
HPC = 4            # heads per core
HG = 256           # head-group width (HPC * DH)
DH = 64
P = 128
NS = S // P        # 16 s-tiles
ND = D // P        # 8 d-tiles
QC = 512           # q-chunk width
NQC = S // QC      # 4 chunks
NPAIR = 2          # head pairs per core
XSH = 512          # x rows shipped per core (B*S/8)

GX = [[0, 1, 2, 3], [4, 5, 6, 7]]           # batch groups
GW = [[0, 4], [1, 5], [2, 6], [3, 7]]       # weight-half pairs

_COMPILED = None
_CACHED = None
_FIRST_DONE = False


# blob row layout (all bf16, 128 cols): x shard rows then weight slices
BX0, BX1 = 0, 4096          # x shard   [512,1024]  -> [4096,128]
BQ0, BQ1 = 4096, 5120       # Wq half   [1024,128]
BK0, BK1 = 5120, 6144       # Wk half
BV0, BV1 = 6144, 7168       # Wv half
BO0, BO1 = 7168, 8192       # Wo half   [128,1024]  -> [1024,128]
NBLOB = 8192


def _emit(nc, tc, bass, mybir, make_identity, blob, outp):
    FR = mybir.dt.float32r
    F32 = mybir.dt.float32
    BF = mybir.dt.bfloat16
    Exp = mybir.ActivationFunctionType.Exp
    mult = mybir.AluOpType.mult
    add = mybir.AluOpType.add
    byp = mybir.AluOpType.bypass

    with (
        tc.tile_pool(name="persist", bufs=1) as pp,
        tc.tile_pool(name="psS", bufs=2, space="PSUM") as psa,
        tc.tile_pool(name="psPV", bufs=2, space="PSUM") as psb,
        tc.tile_pool(name="psO", bufs=2, space="PSUM") as psc,
        tc.tile_pool(name="wpool", bufs=1) as wp,
        tc.tile_pool(name="xcpool", bufs=2) as xcp,
        tc.tile_pool(name="xfpool", bufs=1) as xfp,
        tc.tile_pool(name="xtpool", bufs=2) as xtp,
        tc.tile_pool(name="eppool", bufs=2) as epp,
        tc.tile_pool(name="ctxpool", bufs=2) as cxp,
        tc.tile_pool(name="rpool", bufs=4) as rp,
        tc.tile_pool(name="bcpool", bufs=2) as bcp,
        tc.tile_pool(name="stagepool", bufs=2) as stp,
        tc.tile_pool(name="opool", bufs=2) as obp,
        tc.tile_pool(name="dram", bufs=1, space="DRAM") as dp,
    ):
        # ---- DRAM bounces + collective prologue ----
        # collectives can't touch I/O tensors: one bounce copy of the whole
        # input blob, then gather slices of it. Collectives treat buffers
        # linearly (.opt()), so the declared shapes of in/out only need to
        # agree byte-wise.
        bin_ = dp.tile([NBLOB, P], BF)
        xag = dp.tile([S, D], BF)          # full x[b] after group AllGather
        wqag = dp.tile([2, D, P], BF)      # [half, :, :]
        wkag = dp.tile([2, D, P], BF)
        wvag = dp.tile([2, D, P], BF)
        woag = dp.tile([2, P, D], BF)
        pout = dp.tile([S, D], BF)         # this core's out partial
        rsout = dp.tile([XSH, D], BF)      # reduced shard
        oag = dp.tile([B * S, D], BF)      # full output, replicated

        # bounce on the gpsimd queue: same engine as the collectives, so
        # the AllGathers are hardware-ordered after it
        nc.gpsimd.dma_start(bin_[:], blob[:])
        nc.gpsimd.collective_compute(
            "AllGather", byp, replica_groups=GX,
            ins=[bin_[BX0:BX1, :].opt()], outs=[xag[:].opt()],
        )
        nc.gpsimd.collective_compute(
            "AllGather", byp, replica_groups=GW,
            ins=[bin_[BQ0:BQ1, :].opt()], outs=[wqag[:].opt()],
        )
        nc.gpsimd.collective_compute(
            "AllGather", byp, replica_groups=GW,
            ins=[bin_[BK0:BK1, :].opt()], outs=[wkag[:].opt()],
        )
        nc.gpsimd.collective_compute(
            "AllGather", byp, replica_groups=GW,
            ins=[bin_[BV0:BV1, :].opt()], outs=[wvag[:].opt()],
        )
        nc.gpsimd.collective_compute(
            "AllGather", byp, replica_groups=GW,
            ins=[bin_[BO0:BO1, :].opt()], outs=[woag[:].opt()],
        )

        # persistent tensors
        qt = pp.tile([P, NPAIR, S], FR)        # Q^T pack: parts 0:64 head 2p, 64:128 head 2p+1
        kt = pp.tile([P, NPAIR, S], FR)        # K^T pack
        vv = pp.tile([P, NS, HPC, DH + 1], BF) # V natural per head + ones column
        ident = pp.tile([P, P], FR)
        tri = pp.tile([P, P], BF)              # 1.0 where part(k) <= free(q) else 0

        nc.vector.memset(vv[:, :, :, DH], 1.0)

        x_tiles = {}
        xt_tiles = {}
        ctx_tiles = {}

        def emit_xdma(cc):
            x_c = xcp.tile([P, 4, D], BF, name="x_c")
            for si in range(4):
                s = 4 * cc + si
                nc.gpsimd.dma_start(
                    out=x_c[:, si, :], in_=xag[s * P:(s + 1) * P, :])
            x_tiles[cc] = x_c

        # chunk 0 lands quarter-major in small pieces so the dt=0
        # transposes can start early; data is in flight while the masks
        # and identity build
        x_c = xcp.tile([P, 4, D], BF, name="x_c")
        engs0 = (nc.gpsimd, nc.scalar, nc.gpsimd, nc.scalar)
        q = D // 4
        for si in range(4):
            engs0[si].dma_start(out=x_c[:, si, 0:q],
                                in_=xag[si * P:(si + 1) * P, 0:q])
        x_tiles[0] = x_c
        # memset on float32r trips walrus ISA check; memset via f32 view
        nc.gpsimd.memset(ident[:].bitcast(F32), 0.0)
        make_identity(nc, ident[:], nomemset=True)
        for hh in range(1, 4):
            lo, hi = hh * q, (hh + 1) * q
            for si in range(4):
                engs0[si].dma_start(out=x_c[:, si, lo:hi],
                                    in_=xag[si * P:(si + 1) * P, lo:hi])
        nc.gpsimd.memset(tri[:], 0.0)
        # pred: -1 + p - f >= 0  (p > f) -> keep 0 ; else fill 1.0
        nc.gpsimd.affine_select(
            out=tri[:], in_=tri[:],
            compare_op=mybir.AluOpType.is_ge,
            fill=1.0, base=-1, channel_multiplier=1, pattern=[[-1, P]],
        )

        # weights: bf16 halves land in a staging tile, then one DVE copy
        # converts to the f32r layout the matmuls expect
        wq_sb = wp.tile([P, ND, HG], FR)
        wk_sb = wp.tile([P, ND, HG], FR)
        wv_sb = wp.tile([P, ND, HG], FR)
        wo_sb = wp.tile([P, NPAIR, D], FR)
        wstage = wp.tile([P, ND, HG], BF)
        wostage = wp.tile([P, NPAIR, D], BF)

        def load_w(wag, dst, eng):
            for dt in range(ND):
                for h in range(2):
                    eng.dma_start(
                        out=wstage[:, dt, h * P:(h + 1) * P],
                        in_=wag[h, dt * P:(dt + 1) * P, :],
                    )
            nc.vector.tensor_copy(dst[:], wstage[:])

        load_w(wqag, wq_sb, nc.sync)
        load_w(wkag, wk_sb, nc.sync)
        load_w(wvag, wv_sb, nc.scalar)
        # wo packed by head pair: partitions 0:64 head 2p, 64:128 head 2p+1;
        # pair pr rows = half pr of the gathered slice
        for pr in range(NPAIR):
            nc.sync.dma_start(
                out=wostage[0:DH, pr, :], in_=woag[pr, 0:DH, :])
            nc.sync.dma_start(
                out=wostage[DH:P, pr, :], in_=woag[pr, DH:P, :])
        nc.vector.tensor_copy(wo_sb[:], wostage[:])

        # phase-1 pieces use 1-bank tiles in the psO pool so their allocs
        # never wait on the slow exp drains that pace the psS pool
        def qk_pair(cc, pair):
            xT_c = xt_tiles[cc]
            ps_q = psc.tile([P, QC], F32, name="ps_o")
            for dt in range(ND):
                nc.tensor.matmul(
                    ps_q[:],
                    wq_sb[:, dt, pair * P:(pair + 1) * P],
                    xT_c[:, dt, :],
                    start=(dt == 0), stop=(dt == ND - 1),
                )
            nc.vector.tensor_copy(qt[:, pair, cc * QC:(cc + 1) * QC], ps_q[:])
            ps_k = psc.tile([P, QC], F32, name="ps_o")
            for dt in range(ND):
                nc.tensor.matmul(
                    ps_k[:],
                    wk_sb[:, dt, pair * P:(pair + 1) * P],
                    xT_c[:, dt, :],
                    start=(dt == 0), stop=(dt == ND - 1),
                )
            nc.vector.tensor_copy(kt[:, pair, cc * QC:(cc + 1) * QC], ps_k[:])

        def ph1_pieces(cc):
            def p_transpose():
                x_c = x_tiles.pop(cc)
                # bf16 wire -> f32 on ACT (idle during phase 1); per-si so
                # the dt=0 transposes don't wait on the whole chunk
                x_f = xfp.tile([P, 4, D], FR, name="x_f")
                for si in range(4):
                    nc.vector.tensor_copy(x_f[:, si, :], x_c[:, si, :])
                xT_c = xtp.tile([P, ND, QC], FR, name="xT_c")
                xt_tiles[cc] = xT_c
                for dt in range(ND):
                    ps_t = psc.tile([P, QC], FR, name="ps_o")
                    for si in range(4):
                        nc.tensor.transpose(
                            ps_t[:, si * P:(si + 1) * P],
                            x_f[:, si, dt * P:(dt + 1) * P],
                            ident[:],
                        )
                    nc.vector.tensor_copy(xT_c[:, dt, :], ps_t[:])

            def p_qk0():
                qk_pair(cc, 0)

            def p_qk1():
                qk_pair(cc, 1)

            def p_v():
                xT_c = xt_tiles.pop(cc)
                for si in range(4):
                    ps_v = psc.tile([P, QC], F32, name="ps_o")
                    for dt in range(ND):
                        nc.tensor.matmul(
                            ps_v[:, 0:HG],
                            xT_c[:, dt, si * P:(si + 1) * P],
                            wv_sb[:, dt, :],
                            start=(dt == 0), stop=(dt == ND - 1),
                        )
                    nc.vector.tensor_copy(
                        vv[:, 4 * cc + si, :, 0:DH], ps_v[:, 0:HG]
                    )

            return [p_transpose, p_qk0, p_qk1, p_v]

        def scores_unit_thunks(cc, h, ep):
            T = 4 * cc + 4
            pr = h // 2
            po = DH * (h % 2)
            thunks = []
            t = 0
            while t < T:
                if t + 2 <= 4 * cc:
                    # two full k-tiles share a 2-bank PSUM tile -> one exp
                    def u_pair(t=t):
                        ps_s = psa.tile([P, 2 * QC], F32, name="ps")
                        for uu in range(2):
                            nc.tensor.matmul(
                                ps_s[:, uu * QC:(uu + 1) * QC],
                                kt[po:po + DH, pr, (t + uu) * P:(t + uu + 1) * P],
                                qt[po:po + DH, pr, cc * QC:(cc + 1) * QC],
                                start=True, stop=True,
                            )
                        nc.scalar.activation(
                            ep[:, t * QC:(t + 2) * QC], ps_s[:], Exp, scale=0.125
                        )
                    thunks.append(u_pair)
                    t += 2
                else:
                    # diagonal k-tile: only causally-valid columns
                    jd = t - 4 * cc
                    lo = jd * P if jd > 0 else 0
                    def u_diag(t=t, lo=lo):
                        ps_s = psa.tile([P, 2 * QC], F32, name="ps")
                        nc.tensor.matmul(
                            ps_s[:, lo:QC],
                            kt[po:po + DH, pr, t * P:(t + 1) * P],
                            qt[po:po + DH, pr, cc * QC + lo:(cc + 1) * QC],
                            start=True, stop=True,
                        )
                        nc.scalar.activation(
                            ep[:, t * QC + lo:(t + 1) * QC], ps_s[:, lo:QC],
                            Exp, scale=0.125,
                        )
                    thunks.append(u_diag)
                    t += 1
            return thunks

        def tri_fixups(cc, ep):
            # causal fixups on the 4 diagonal k-tiles (cols < jd*P are
            # never read: PV matmuls are col-trimmed the same way)
            for jd in range(4):
                t2 = 4 * cc + jd
                base = t2 * QC + jd * P
                nc.vector.tensor_tensor(
                    ep[:, base:base + P], ep[:, base:base + P], tri[:], op=mult
                )

        def pv_thunks(cc, h, ep, ps_ctx):
            T = 4 * cc + 4
            thunks = []
            for t in range(T):
                jd = t - 4 * cc
                lo = jd * P if jd > 0 else 0
                def u(t=t, lo=lo):
                    nc.tensor.matmul(
                        ps_ctx[:, lo:QC],
                        vv[:, t, h, :],
                        ep[:, t * QC + lo:(t + 1) * QC],
                        start=(t == 0), stop=(t == T - 1),
                    )
                thunks.append(u)
            return thunks

        def emit_pv_finish(cc, h, ps_ctx, recip):
            ctx_c = ctx_tiles[cc]
            # broadcast recip across 64 partitions on the Pool engine
            # (SBUF->SBUF; tensor_tensor may read only one PSUM input)
            bc_sb = bcp.tile([DH, QC], F32, name="bc_sb")
            nc.gpsimd.partition_broadcast(bc_sb[:], recip[:])
            pr, odd = divmod(h, 2)
            if odd == 0:
                nc.vector.tensor_tensor(
                    ctx_c[0:DH, pr, :], ps_ctx[0:DH, :], bc_sb[:], op=mult
                )
            else:
                # odd head lands on partitions 64:128 via SBUF->SBUF DMA
                stage = stp.tile([DH, QC], FR, name="stage")
                nc.vector.tensor_tensor(
                    stage[:], ps_ctx[0:DH, :], bc_sb[:], op=mult
                )
                nc.gpsimd.dma_start(out=ctx_c[DH:P, pr, :], in_=stage[:])

        def emit_outproj(cc, last=False):
            ctx_c = ctx_tiles.pop(cc)
            # all partial stores go on the gpsimd queue: the ReduceScatter
            # that consumes pout runs there too, so it is hardware-ordered
            # after every store (device exec is not the wall-clock
            # bottleneck; transfer is)
            for jq in range(4):
                i = 4 * cc + jq
                out_sb = obp.tile([P, D], BF)
                for nk in range(2):
                    ps_o = psc.tile([P, QC], F32)
                    for pr in range(NPAIR):
                        nc.tensor.matmul(
                            ps_o[:],
                            ctx_c[:, pr, jq * P:(jq + 1) * P],
                            wo_sb[:, pr, nk * QC:(nk + 1) * QC],
                            start=(pr == 0), stop=(pr == NPAIR - 1),
                        )
                    nc.vector.tensor_copy(out_sb[:, nk * QC:(nk + 1) * QC], ps_o[:])
                    nc.gpsimd.dma_start(
                        out=pout[i * P:(i + 1) * P,
                                 nk * QC:(nk + 1) * QC],
                        in_=out_sb[:, nk * QC:(nk + 1) * QC],
                    )

        # ---- driver: chunk-interleaved software pipeline ----
        # Per head-block: scores(h) psa units are ACT-paced; PV(h-1)
        # chain matmuls are interleaved between them so the PE FIFO
        # always has runnable work while an exp drains a psa buffer.
        prev = [None]
        nfin = {0: 0, 1: 0, 2: 0, 3: 0}
        # last finish of each chunk is an even head: no Pool shift on
        # the critical tail before outproj
        HEAD_ORDER = (1, 0, 3, 2)

        def head_block(cc, h, piece):
            if cc not in ctx_tiles:
                ctx_tiles[cc] = cxp.tile([P, NPAIR, QC], FR, name="ctx_c")
            ep = epp.tile([P, NS * QC], BF, name="ep")
            su = scores_unit_thunks(cc, h, ep)
            pvt, fin = [], None
            if prev[0] is not None:
                pcc, ph2, pep = prev[0]
                ps_ctx = psb.tile([DH + 1, QC], F32, name="pv")
                pvt = pv_thunks(pcc, ph2, pep, ps_ctx)
                fin = (pcc, ph2, ps_ctx)
            su[0]()
            if len(su) > 1:
                su[1]()
            rest = su[2:]
            nslots = len(rest) + 1
            done = 0
            for j in range(nslots):
                want = ((j + 1) * len(pvt)) // nslots
                while done < want:
                    pvt[done]()
                    done += 1
                if j < len(rest):
                    rest[j]()
            # pv_finish goes on the DVE queue ahead of the fixups so the
            # psb slot frees before the block-end DVE burst
            ofin = None
            if fin is not None:
                recip = rp.tile([1, QC], F32)
                nc.vector.reciprocal(recip[:], fin[2][DH:DH + 1, :])
                pcc, ph2, ps_ctx = fin
                emit_pv_finish(pcc, ph2, ps_ctx, recip)
                nfin[pcc] += 1
                if nfin[pcc] == HPC:
                    ofin = pcc
            tri_fixups(cc, ep)
            if piece is not None:
                piece()
            if ofin is not None:
                emit_outproj(ofin)
            prev[0] = (cc, h, ep)

        def attn(cc, pieces=()):
            it = iter(pieces)
            for h in HEAD_ORDER:
                head_block(cc, h, next(it, None))

        emit_xdma(1)
        for p in ph1_pieces(0):
            p()
        emit_xdma(2)
        for p in ph1_pieces(1):
            p()
        emit_xdma(3)
        attn(0, ph1_pieces(2))
        attn(1, ph1_pieces(3))
        attn(3)
        attn(2)
        # flush the last head
        pcc, ph2, pep = prev[0]
        ps_ctx = psb.tile([DH + 1, QC], F32, name="pv")
        for u in pv_thunks(pcc, ph2, pep, ps_ctx):
            u()
        recip = rp.tile([1, QC], F32)
        nc.vector.reciprocal(recip[:], ps_ctx[DH:DH + 1, :])
        emit_pv_finish(pcc, ph2, ps_ctx, recip)
        emit_outproj(pcc, last=True)

        # ---- epilogue: sum the 4 head-group partials on device, then
        # gather the full output on every core so the host fetches it
        # from a single device ----
        nc.gpsimd.collective_compute(
            "ReduceScatter", add, replica_groups=GX,
            ins=[pout[:].opt()], outs=[rsout[:].opt()],
        )
        nc.gpsimd.collective_compute(
            "AllGather", byp, replica_groups=[list(range(8))],
            ins=[rsout[:].opt()], outs=[oag[:].opt()],
        )
        nc.gpsimd.dma_start(out=outp[:], in_=oag[:])


def _build():
    import concourse.bass as bass
    import concourse.tile as tile
    from concourse import bacc, mybir
    from concourse.masks import make_identity

    BF = mybir.dt.bfloat16

    nc = bacc.Bacc(
        "TRN2", target_bir_lowering=False, debug=False,
        enable_asserts=True, num_devices=8,
    )
    blob = nc.dram_tensor("blob", [NBLOB, P], BF, kind="ExternalInput")
    outp = nc.dram_tensor("outp", [B * S, D], BF, kind="ExternalOutput")

    with tile.TileContext(nc) as tc:
        _emit(nc, tc, bass, mybir, make_identity, blob, outp)
    nc.compile()
    return nc


def _get_compiled():
    global _COMPILED
    if _COMPILED is None:
        _COMPILED = _build()
    return _COMPILED


def _to_u16(a):
    """f32 -> bf16 bits (round-to-nearest, ties away) as uint16."""
    u = np.ascontiguousarray(a, dtype=np.float32).view(np.uint32)
    return ((u + 0x8000) >> 16).astype(np.uint16)


def _from_bf16_f32(a):
    """bf16 (or uint16 view) -> exact f32."""
    u = np.asarray(a).view(np.uint16)
    return (u.astype(np.uint32) << 16).view(np.float32)


def _has_nan_bf16(a):
    """True if any bf16 element is nan/inf (transient device fault sign)."""
    u = np.asarray(a).view(np.uint16)
    return bool(((u & 0x7FFF) >= 0x7F80).any())


# core c (b = c//4, g = c%4) ships half b of head-group g's slice; the
# slice at column offset g*HG + b*P is 128-column block j = 2g + b
_PERMW = np.array([(c % 4) * 2 + (c // 4) for c in range(8)])
_BLOB_SCRATCH = None


def _globals_from_inputs(x, Wq, Wk, Wv, Wo):
    """One packed (8*NBLOB, 128) bf16 blob, core-major, matching the
    shard_map layout run_bass_via_pjrt uses."""
    global _BLOB_SCRATCH
    if _BLOB_SCRATCH is None:
        _BLOB_SCRATCH = np.empty((8, NBLOB, P), np.uint16)
    blob = _BLOB_SCRATCH
    blob[:, BX0:BX1, :] = _to_u16(
        np.asarray(x, np.float32).reshape(B * S, D)).reshape(8, BX1 - BX0, P)
    blob[:, BQ0:BQ1, :] = _to_u16(Wq).reshape(D, 8, P).transpose(1, 0, 2)[_PERMW]
    blob[:, BK0:BK1, :] = _to_u16(Wk).reshape(D, 8, P).transpose(1, 0, 2)[_PERMW]
    blob[:, BV0:BV1, :] = _to_u16(Wv).reshape(D, 8, P).transpose(1, 0, 2)[_PERMW]
    blob[:, BO0:BO1, :] = _to_u16(Wo).reshape(8, P, D)[_PERMW].reshape(
        8, BO1 - BO0, P)
    return {"blob": blob.reshape(8 * NBLOB, P).view(ml_dtypes.bfloat16)}


def _finalize(out_bf_flat, bo):
    out = _from_bf16_f32(out_bf_flat).reshape(B, S, D)
    bo32 = np.asarray(bo, np.float32)
    if bo32.any():
        out += bo32[None, None, :]
    return out


def _in_maps(gl):
    return [{"blob": gl["blob"][c * NBLOB:(c + 1) * NBLOB]} for c in range(8)]


def _get_cached_runner():
    """jit-compiled shard_map over the 8 cores, built once and reused.

    Mirrors bass2jax.run_bass_via_pjrt exactly (same primitive, same NEFF,
    same donation scheme) but keeps the jitted callable alive so warm calls
    skip re-tracing/re-lowering."""
    global _CACHED
    if _CACHED is not None:
        return _CACHED
    import jax
    from jax.sharding import Mesh, PartitionSpec
    from jax.experimental.shard_map import shard_map
    from concourse import mybir
    from concourse.bass2jax import (
        _bass_exec_p, install_neuronx_cc_hook, partition_id_tensor,
    )

    nc = _get_compiled()
    install_neuronx_cc_hook()
    partition_name = nc.partition_id_tensor.name if nc.partition_id_tensor else None
    in_names = []
    out_names = []
    out_avals = []
    out_shapes = []
    for alloc in nc.m.functions[0].allocations:
        if not isinstance(alloc, mybir.MemoryLocationSet):
            continue
        name = alloc.memorylocations[0].name
        if alloc.kind == "ExternalInput":
            if name != partition_name:
                in_names.append(name)
        elif alloc.kind == "ExternalOutput":
            shape = tuple(alloc.tensor_shape)
            dtype = mybir.dt.np(alloc.dtype)
            out_names.append(name)
            out_avals.append(jax.core.ShapedArray(shape, dtype))
            out_shapes.append((shape, dtype))
    n_params = len(in_names)
    # no donated zero buffers: the kernel writes every output element, so
    # outputs may start uninitialized and nothing extra goes over the wire
    in_names_all = list(in_names)
    if partition_name is not None:
        in_names_all.append(partition_name)

    def _body(*args):
        operands = list(args)
        if partition_name is not None:
            operands.append(partition_id_tensor())
        outs = _bass_exec_p.bind(
            *operands,
            out_avals=tuple(out_avals),
            in_names=tuple(in_names_all),
            out_names=tuple(out_names),
            lowering_input_output_aliases=(),
            sim_require_finite=True,
            sim_require_nnan=True,
            nc=nc,
        )
        return tuple(outs)

    devices = jax.devices()[:8]
    mesh = Mesh(np.asarray(devices), ("core",))
    # inputs are sharded per core; the output is replicated (the kernel
    # AllGathers it) so the host fetches it from one device only
    in_specs = (PartitionSpec("core"),) * n_params
    out_specs = (PartitionSpec(),) * len(out_names)
    sharded = jax.jit(
        shard_map(_body, mesh=mesh, in_specs=in_specs, out_specs=out_specs,
                  check_rep=False),
        keep_unused=True,
    )
    _CACHED = (sharded, in_names, out_names, out_shapes)
    return _CACHED


def run_spmd(x, Wq, Wk, Wv, Wo, bo, **spmd_kwargs):
    """Run the 8-core kernel; returns (full_output, BassKernelResults|None)."""
    global _FIRST_DONE
    gl = _globals_from_inputs(x, Wq, Wk, Wv, Wo)

    if spmd_kwargs or not _FIRST_DONE:
        # first (compile) call and trace/debug calls go through the stock
        # runner; warm calls reuse the jitted executable below
        from concourse.bass_utils import run_bass_kernel_spmd
        nc = _get_compiled()
        try:
            res = run_bass_kernel_spmd(nc, _in_maps(gl), list(range(8)),
                                       **spmd_kwargs)
        except Exception:
            if spmd_kwargs:
                raise
            # transient device wedge (NRT_EXEC_UNIT_UNRECOVERABLE etc.):
            # one retry
            res = run_bass_kernel_spmd(nc, _in_maps(gl), list(range(8)))
        _FIRST_DONE = True
        # warm the cached runner (trace/lower/XLA-compile) during the
        # cold call so the first timed warm call doesn't pay for it
        try:
            sharded, in_names, out_names, _ = _get_cached_runner()
            arrs = sharded(*[gl[name] for name in in_names])
            np.asarray(arrs[0])
        except Exception:
            pass
        # output is replicated across cores; take core 0's copy
        out_flat = res.results[0]["outp"]
        if not spmd_kwargs and _has_nan_bf16(out_flat):
            # transient device fault: rerun once
            res = run_bass_kernel_spmd(nc, _in_maps(gl), list(range(8)))
            out_flat = res.results[0]["outp"]
        out = _finalize(out_flat, bo)
        return out, res

    try:
        sharded, in_names, out_names, out_shapes = _get_cached_runner()
        out_arrs = sharded(*[gl[name] for name in in_names])
        out_flat = np.asarray(out_arrs[out_names.index("outp")])
        if _has_nan_bf16(out_flat):
            raise RuntimeError("nan in kernel output (transient fault)")
    except Exception:
        # recover from transient device failures via the stock runner
        from concourse.bass_utils import run_bass_kernel_spmd
        res = run_bass_kernel_spmd(_get_compiled(), _in_maps(gl),
                                   list(range(8)))
        out_flat = res.results[0]["outp"]
    return _finalize(out_flat, bo), None


def kernel(x, Wq, Wk, Wv, Wo, bo):
    out, _ = run_spmd(x, Wq, Wk, Wv, Wo, bo)
    return out


# revision 11
# speedup vs baseline: 1.1087x; 1.1087x over previous
import sys

if "/opt/trn_rl_repo" not in sys.path:
    sys.path.insert(0, "/opt/trn_rl_repo")

import numpy as np
import ml_dtypes

B, S, D, H = 2, 2048, 1024, 16
HPC = 4            # heads per core
HG = 256           # head-group width (HPC * DH)
DH = 64
P = 128
NS = S // P        # 16 s-tiles
ND = D // P        # 8 d-tiles
QC = 512           # q-chunk width
NQC = S // QC      # 4 chunks
NPAIR = 2          # head pairs per core
XSH = 512          # x rows shipped per core (B*S/8)

GX = [[0, 1, 2, 3], [4, 5, 6, 7]]           # batch groups
GW = [[0, 4], [1, 5], [2, 6], [3, 7]]       # weight-half pairs

_COMPILED = None
_CACHED = None
_FIRST_DONE = False


# blob row layout (all bf16, 128 cols): x shard rows then weight slices
BX0, BX1 = 0, 4096          # x shard   [512,1024]  -> [4096,128]
BQ0, BQ1 = 4096, 5120       # Wq half   [1024,128]
BK0, BK1 = 5120, 6144       # Wk half
BV0, BV1 = 6144, 7168       # Wv half
BO0, BO1 = 7168, 8192       # Wo half   [128,1024]  -> [1024,128]
NBLOB = 8192


def _emit(nc, tc, bass, mybir, make_identity, blob, outp):
    FR = mybir.dt.float32r
    F32 = mybir.dt.float32
    BF = mybir.dt.bfloat16
    Exp = mybir.ActivationFunctionType.Exp
    mult = mybir.AluOpType.mult
    add = mybir.AluOpType.add
    byp = mybir.AluOpType.bypass

    with (
        tc.tile_pool(name="persist", bufs=1) as pp,
        tc.tile_pool(name="psS", bufs=2, space="PSUM") as psa,
        tc.tile_pool(name="psPV", bufs=2, space="PSUM") as psb,
        tc.tile_pool(name="psO", bufs=2, space="PSUM") as psc,
        tc.tile_pool(name="wpool", bufs=1) as wp,
        tc.tile_pool(name="xcpool", bufs=2) as xcp,
        tc.tile_pool(name="xfpool", bufs=1) as xfp,
        tc.tile_pool(name="xtpool", bufs=2) as xtp,
        tc.tile_pool(name="eppool", bufs=2) as epp,
        tc.tile_pool(name="ctxpool", bufs=2) as cxp,
        tc.tile_pool(name="rpool", bufs=4) as rp,
        tc.tile_pool(name="bcpool", bufs=2) as bcp,
        tc.tile_pool(name="stagepool", bufs=2) as stp,
        tc.tile_pool(name="opool", bufs=2) as obp,
        tc.tile_pool(name="dram", bufs=1, space="DRAM") as dp,
    ):
        # ---- DRAM bounces + collective prologue ----
        # collectives can't touch I/O tensors: one bounce copy of the whole
        # input blob, then gather slices of it. Collectives treat buffers
        # linearly (.opt()), so the declared shapes of in/out only need to
        # agree byte-wise.
        bin_ = dp.tile([NBLOB, P], BF)
        xag = dp.tile([S, D], BF)          # full x[b] after group AllGather
        wqag = dp.tile([2, D, P], BF)      # [half, :, :]
        wkag = dp.tile([2, D, P], BF)
        wvag = dp.tile([2, D, P], BF)
        woag = dp.tile([2, P, D], BF)
        pout = dp.tile([S, D], BF)         # this core's out partial
        rsout = dp.tile([XSH, D], BF)      # reduced shard
        oag = dp.tile([B * S, D], BF)      # full output, replicated

        # bounce on the gpsimd queue: same engine as the collectives, so
        # the AllGathers are hardware-ordered after it
        nc.gpsimd.dma_start(bin_[:], blob[:])
        nc.gpsimd.collective_compute(
            "AllGather", byp, replica_groups=GX,
            ins=[bin_[BX0:BX1, :].opt()], outs=[xag[:].opt()],
        )
        nc.gpsimd.collective_compute(
            "AllGather", byp, replica_groups=GW,
            ins=[bin_[BQ0:BQ1, :].opt()], outs=[wqag[:].opt()],
        )
        nc.gpsimd.collective_compute(
            "AllGather", byp, replica_groups=GW,
            ins=[bin_[BK0:BK1, :].opt()], outs=[wkag[:].opt()],
        )
        nc.gpsimd.collective_compute(
            "AllGather", byp, replica_groups=GW,
            ins=[bin_[BV0:BV1, :].opt()], outs=[wvag[:].opt()],
        )
        nc.gpsimd.collective_compute(
            "AllGather", byp, replica_groups=GW,
            ins=[bin_[BO0:BO1, :].opt()], outs=[woag[:].opt()],
        )

        # persistent tensors
        qt = pp.tile([P, NPAIR, S], FR)        # Q^T pack: parts 0:64 head 2p, 64:128 head 2p+1
        kt = pp.tile([P, NPAIR, S], FR)        # K^T pack
        vv = pp.tile([P, NS, HPC, DH + 1], BF) # V natural per head + ones column
        ident = pp.tile([P, P], FR)
        tri = pp.tile([P, P], BF)              # 1.0 where part(k) <= free(q) else 0

        nc.vector.memset(vv[:, :, :, DH], 1.0)

        x_tiles = {}
        xt_tiles = {}
        ctx_tiles = {}

        def emit_xdma(cc):
            x_c = xcp.tile([P, 4, D], BF, name="x_c")
            for si in range(4):
                s = 4 * cc + si
                nc.gpsimd.dma_start(
                    out=x_c[:, si, :], in_=xag[s * P:(s + 1) * P, :])
            x_tiles[cc] = x_c

        # chunk 0 lands quarter-major in small pieces so the dt=0
        # transposes can start early; data is in flight while the masks
        # and identity build
        x_c = xcp.tile([P, 4, D], BF, name="x_c")
        engs0 = (nc.gpsimd, nc.scalar, nc.gpsimd, nc.scalar)
        q = D // 4
        for si in range(4):
            engs0[si].dma_start(out=x_c[:, si, 0:q],
                                in_=xag[si * P:(si + 1) * P, 0:q])
        x_tiles[0] = x_c
        # memset on float32r trips walrus ISA check; memset via f32 view
        nc.gpsimd.memset(ident[:].bitcast(F32), 0.0)
        make_identity(nc, ident[:], nomemset=True)
        for hh in range(1, 4):
            lo, hi = hh * q, (hh + 1) * q
            for si in range(4):
                engs0[si].dma_start(out=x_c[:, si, lo:hi],
                                    in_=xag[si * P:(si + 1) * P, lo:hi])
        nc.gpsimd.memset(tri[:], 0.0)
        # pred: -1 + p - f >= 0  (p > f) -> keep 0 ; else fill 1.0
        nc.gpsimd.affine_select(
            out=tri[:], in_=tri[:],
            compare_op=mybir.AluOpType.is_ge,
            fill=1.0, base=-1, channel_multiplier=1, pattern=[[-1, P]],
        )

        # weights: bf16 halves land in a staging tile, then one DVE copy
        # converts to the f32r layout the matmuls expect
        wq_sb = wp.tile([P, ND, HG], FR)
        wk_sb = wp.tile([P, ND, HG], FR)
        wv_sb = wp.tile([P, ND, HG], FR)
        wo_sb = wp.tile([P, NPAIR, D], FR)
        wstage = wp.tile([P, ND, HG], BF)
        wostage = wp.tile([P, NPAIR, D], BF)

        def load_w(wag, dst, eng):
            for dt in range(ND):
                for h in range(2):
                    eng.dma_start(
                        out=wstage[:, dt, h * P:(h + 1) * P],
                        in_=wag[h, dt * P:(dt + 1) * P, :],
                    )
            nc.vector.tensor_copy(dst[:], wstage[:])

        load_w(wqag, wq_sb, nc.sync)
        load_w(wkag, wk_sb, nc.sync)
        load_w(wvag, wv_sb, nc.scalar)
        # wo packed by head pair: partitions 0:64 head 2p, 64:128 head 2p+1;
        # pair pr rows = half pr of the gathered slice
        for pr in range(NPAIR):
            nc.sync.dma_start(
                out=wostage[0:DH, pr, :], in_=woag[pr, 0:DH, :])
            nc.sync.dma_start(
                out=wostage[DH:P, pr, :], in_=woag[pr, DH:P, :])
        nc.vector.tensor_copy(wo_sb[:], wostage[:])

        # phase-1 pieces use 1-bank tiles in the psO pool so their allocs
        # never wait on the slow exp drains that pace the psS pool
        def qk_pair(cc, pair):
            xT_c = xt_tiles[cc]
            ps_q = psc.tile([P, QC], F32, name="ps_o")
            for dt in range(ND):
                nc.tensor.matmul(
                    ps_q[:],
                    wq_sb[:, dt, pair * P:(pair + 1) * P],
                    xT_c[:, dt, :],
                    start=(dt == 0), stop=(dt == ND - 1),
                )
            nc.vector.tensor_copy(qt[:, pair, cc * QC:(cc + 1) * QC], ps_q[:])
            ps_k = psc.tile([P, QC], F32, name="ps_o")
            for dt in range(ND):
                nc.tensor.matmul(
                    ps_k[:],
                    wk_sb[:, dt, pair * P:(pair + 1) * P],
                    xT_c[:, dt, :],
                    start=(dt == 0), stop=(dt == ND - 1),
                )
            nc.vector.tensor_copy(kt[:, pair, cc * QC:(cc + 1) * QC], ps_k[:])

        def ph1_pieces(cc):
            def p_transpose():
                x_c = x_tiles.pop(cc)
                # bf16 wire -> f32 on ACT (idle during phase 1); per-si so
                # the dt=0 transposes don't wait on the whole chunk
                x_f = xfp.tile([P, 4, D], FR, name="x_f")
                for si in range(4):
                    nc.vector.tensor_copy(x_f[:, si, :], x_c[:, si, :])
                xT_c = xtp.tile([P, ND, QC], FR, name="xT_c")
                xt_tiles[cc] = xT_c
                for dt in range(ND):
                    ps_t = psc.tile([P, QC], FR, name="ps_o")
                    for si in range(4):
                        nc.tensor.transpose(
                            ps_t[:, si * P:(si + 1) * P],
                            x_f[:, si, dt * P:(dt + 1) * P],
                            ident[:],
                        )
                    nc.vector.tensor_copy(xT_c[:, dt, :], ps_t[:])

            def p_qk0():
                qk_pair(cc, 0)

            def p_qk1():
                qk_pair(cc, 1)

            def p_v():
                xT_c = xt_tiles.pop(cc)
                for si in range(4):
                    ps_v = psc.tile([P, QC], F32, name="ps_o")
                    for dt in range(ND):
                        nc.tensor.matmul(
                            ps_v[:, 0:HG],
                            xT_c[:, dt, si * P:(si + 1) * P],
                            wv_sb[:, dt, :],
                            start=(dt == 0), stop=(dt == ND - 1),
                        )
                    nc.vector.tensor_copy(
                        vv[:, 4 * cc + si, :, 0:DH], ps_v[:, 0:HG]
                    )

            return [p_transpose, p_qk0, p_qk1, p_v]

        def scores_unit_thunks(cc, h, ep):
            T = 4 * cc + 4
            pr = h // 2
            po = DH * (h % 2)
            thunks = []
            t = 0
            while t < T:
                if t + 2 <= 4 * cc:
                    # two full k-tiles share a 2-bank PSUM tile -> one exp
                    def u_pair(t=t):
                        ps_s = psa.tile([P, 2 * QC], F32, name="ps")
                        for uu in range(2):
                            nc.tensor.matmul(
                                ps_s[:, uu * QC:(uu + 1) * QC],
                                kt[po:po + DH, pr, (t + uu) * P:(t + uu + 1) * P],
                                qt[po:po + DH, pr, cc * QC:(cc + 1) * QC],
                                start=True, stop=True,
                            )
                        nc.scalar.activation(
                            ep[:, t * QC:(t + 2) * QC], ps_s[:], Exp, scale=0.125
                        )
                    thunks.append(u_pair)
                    t += 2
                else:
                    # diagonal k-tile: only causally-valid columns
                    jd = t - 4 * cc
                    lo = jd * P if jd > 0 else 0
                    def u_diag(t=t, lo=lo):
                        ps_s = psa.tile([P, 2 * QC], F32, name="ps")
                        nc.tensor.matmul(
                            ps_s[:, lo:QC],
                            kt[po:po + DH, pr, t * P:(t + 1) * P],
                            qt[po:po + DH, pr, cc * QC + lo:(cc + 1) * QC],
                            start=True, stop=True,
                        )
                        nc.scalar.activation(
                            ep[:, t * QC + lo:(t + 1) * QC], ps_s[:, lo:QC],
                            Exp, scale=0.125,
                        )
                    thunks.append(u_diag)
                    t += 1
            return thunks

        def tri_fixups(cc, ep):
            # causal fixups on the 4 diagonal k-tiles (cols < jd*P are
            # never read: PV matmuls are col-trimmed the same way)
            for jd in range(4):
                t2 = 4 * cc + jd
                base = t2 * QC + jd * P
                nc.vector.tensor_tensor(
                    ep[:, base:base + P], ep[:, base:base + P], tri[:], op=mult
                )

        def pv_thunks(cc, h, ep, ps_ctx):
            T = 4 * cc + 4
            thunks = []
            for t in range(T):
                jd = t - 4 * cc
                lo = jd * P if jd > 0 else 0
                def u(t=t, lo=lo):
                    nc.tensor.matmul(
                        ps_ctx[:, lo:QC],
                        vv[:, t, h, :],
                        ep[:, t * QC + lo:(t + 1) * QC],
                        start=(t == 0), stop=(t == T - 1),
                    )
                thunks.append(u)
            return thunks

        def emit_pv_finish(cc, h, ps_ctx, recip):
            ctx_c = ctx_tiles[cc]
            # broadcast recip across 64 partitions on the Pool engine
            # (SBUF->SBUF; tensor_tensor may read only one PSUM input)
            bc_sb = bcp.tile([DH, QC], F32, name="bc_sb")
            nc.gpsimd.partition_broadcast(bc_sb[:], recip[:])
            pr, odd = divmod(h, 2)
            if odd == 0:
                nc.vector.tensor_tensor(
                    ctx_c[0:DH, pr, :], ps_ctx[0:DH, :], bc_sb[:], op=mult
                )
            else:
                # odd head lands on partitions 64:128 via SBUF->SBUF DMA
                stage = stp.tile([DH, QC], FR, name="stage")
                nc.vector.tensor_tensor(
                    stage[:], ps_ctx[0:DH, :], bc_sb[:], op=mult
                )
                nc.gpsimd.dma_start(out=ctx_c[DH:P, pr, :], in_=stage[:])

        def emit_outproj(cc, last=False):
            ctx_c = ctx_tiles.pop(cc)
            # all partial stores go on the gpsimd queue: the ReduceScatter
            # that consumes pout runs there too, so it is hardware-ordered
            # after every store (device exec is not the wall-clock
            # bottleneck; transfer is)
            for jq in range(4):
                i = 4 * cc + jq
                out_sb = obp.tile([P, D], BF)
                for nk in range(2):
                    ps_o = psc.tile([P, QC], F32)
                    for pr in range(NPAIR):
                        nc.tensor.matmul(
                            ps_o[:],
                            ctx_c[:, pr, jq * P:(jq + 1) * P],
                            wo_sb[:, pr, nk * QC:(nk + 1) * QC],
                            start=(pr == 0), stop=(pr == NPAIR - 1),
                        )
                    nc.vector.tensor_copy(out_sb[:, nk * QC:(nk + 1) * QC], ps_o[:])
                    nc.gpsimd.dma_start(
                        out=pout[i * P:(i + 1) * P,
                                 nk * QC:(nk + 1) * QC],
                        in_=out_sb[:, nk * QC:(nk + 1) * QC],
                    )

        # ---- driver: chunk-interleaved software pipeline ----
        # Per head-block: scores(h) psa units are ACT-paced; PV(h-1)
        # chain matmuls are interleaved between them so the PE FIFO
        # always has runnable work while an exp drains a psa buffer.
        prev = [None]
        nfin = {0: 0, 1: 0, 2: 0, 3: 0}
        # last finish of each chunk is an even head: no Pool shift on
        # the critical tail before outproj
        HEAD_ORDER = (1, 0, 3, 2)

        def head_block(cc, h, piece):
            if cc not in ctx_tiles:
                ctx_tiles[cc] = cxp.tile([P, NPAIR, QC], FR, name="ctx_c")
            ep = epp.tile([P, NS * QC], BF, name="ep")
            su = scores_unit_thunks(cc, h, ep)
            pvt, fin = [], None
            if prev[0] is not None:
                pcc, ph2, pep = prev[0]
                ps_ctx = psb.tile([DH + 1, QC], F32, name="pv")
                pvt = pv_thunks(pcc, ph2, pep, ps_ctx)
                fin = (pcc, ph2, ps_ctx)
            su[0]()
            if len(su) > 1:
                su[1]()
            rest = su[2:]
            nslots = len(rest) + 1
            done = 0
            for j in range(nslots):
                want = ((j + 1) * len(pvt)) // nslots
                while done < want:
                    pvt[done]()
                    done += 1
                if j < len(rest):
                    rest[j]()
            # pv_finish goes on the DVE queue ahead of the fixups so the
            # psb slot frees before the block-end DVE burst
            ofin = None
            if fin is not None:
                recip = rp.tile([1, QC], F32)
                nc.vector.reciprocal(recip[:], fin[2][DH:DH + 1, :])
                pcc, ph2, ps_ctx = fin
                emit_pv_finish(pcc, ph2, ps_ctx, recip)
                nfin[pcc] += 1
                if nfin[pcc] == HPC:
                    ofin = pcc
            tri_fixups(cc, ep)
            if piece is not None:
                piece()
            if ofin is not None:
                emit_outproj(ofin)
            prev[0] = (cc, h, ep)

        def attn(cc, pieces=()):
            it = iter(pieces)
            for h in HEAD_ORDER:
                head_block(cc, h, next(it, None))

        emit_xdma(1)
        for p in ph1_pieces(0):
            p()
        emit_xdma(2)
        for p in ph1_pieces(1):
            p()
        emit_xdma(3)
        attn(0, ph1_pieces(2))
        attn(1, ph1_pieces(3))
        attn(3)
        attn(2)
        # flush the last head
        pcc, ph2, pep = prev[0]
        ps_ctx = psb.tile([DH + 1, QC], F32, name="pv")
        for u in pv_thunks(pcc, ph2, pep, ps_ctx):
            u()
        recip = rp.tile([1, QC], F32)
        nc.vector.reciprocal(recip[:], ps_ctx[DH:DH + 1, :])
        emit_pv_finish(pcc, ph2, ps_ctx, recip)
        emit_outproj(pcc, last=True)

        # ---- epilogue: sum the 4 head-group partials on device, then
        # gather the full output on every core so the host fetches it
        # from a single device ----
        nc.gpsimd.collective_compute(
            "ReduceScatter", add, replica_groups=GX,
            ins=[pout[:].opt()], outs=[rsout[:].opt()],
        )
        nc.gpsimd.collective_compute(
            "AllGather", byp, replica_groups=[list(range(8))],
            ins=[rsout[:].opt()], outs=[oag[:].opt()],
        )
        nc.gpsimd.dma_start(out=outp[:], in_=oag[:])


def _build():
    import concourse.bass as bass
    import concourse.tile as tile
    from concourse import bacc, mybir
    from concourse.masks import make_identity

    BF = mybir.dt.bfloat16

    nc = bacc.Bacc(
        "TRN2", target_bir_lowering=False, debug=False,
        enable_asserts=True, num_devices=8,
    )
    blob = nc.dram_tensor("blob", [NBLOB, P], BF, kind="ExternalInput")
    outp = nc.dram_tensor("outp", [B * S, D], BF, kind="ExternalOutput")

    with tile.TileContext(nc) as tc:
        _emit(nc, tc, bass, mybir, make_identity, blob, outp)
    nc.compile()
    return nc


def _get_compiled():
    global _COMPILED
    if _COMPILED is None:
        _COMPILED = _build()
    return _COMPILED


def _to_u16(a):
    """f32 -> bf16 bits (round-to-nearest, ties away) as uint16."""
    u = np.ascontiguousarray(a, dtype=np.float32).view(np.uint32)
    return ((u + 0x8000) >> 16).astype(np.uint16)


def _from_bf16_f32(a):
    """bf16 (or uint16 view) -> exact f32."""
    u = np.asarray(a).view(np.uint16)
    return (u.astype(np.uint32) << 16).view(np.float32)


def _has_nan_bf16(a):
    """True if any bf16 element is nan/inf (transient device fault sign)."""
    u = np.asarray(a).view(np.uint16)
    return bool(((u & 0x7FFF) >= 0x7F80).any())


# core c (b = c//4, g = c%4) ships half b of head-group g's slice; the
# slice at column offset g*HG + b*P is 128-column block j = 2g + b
_PERMW = np.array([(c % 4) * 2 + (c // 4) for c in range(8)])
_BLOB_SCRATCH = None
_XSCR = None
_WSCR = None
_POOL = None


def _get_pool():
    global _POOL
    if _POOL is None:
        from concurrent.futures import ThreadPoolExecutor
        _POOL = ThreadPoolExecutor(max_workers=8)
    return _POOL


def _globals_from_inputs(x, Wq, Wk, Wv, Wo):
    """One packed (8*NBLOB, 128) bf16 blob, core-major, matching the
    shard_map layout run_bass_via_pjrt uses. Conversion runs on a thread
    pool with preallocated scratch (numpy ufuncs release the GIL)."""
    global _BLOB_SCRATCH, _XSCR, _WSCR
    if _BLOB_SCRATCH is None:
        _BLOB_SCRATCH = np.empty((8, NBLOB, P), np.uint16)
        _XSCR = [np.empty((XSH, D), np.uint32) for _ in range(8)]
        _WSCR = [np.empty((D, D), np.uint32) for _ in range(4)]
    blob = _BLOB_SCRATCH
    xf = np.ascontiguousarray(x, np.float32).reshape(B * S, D)

    def xtask(c):
        s = _XSCR[c]
        np.add(xf[c * XSH:(c + 1) * XSH].view(np.uint32), 0x8000, out=s)
        np.right_shift(s, 16, out=s)
        blob[c, BX0:BX1, :].reshape(XSH, D)[:] = s

    def wtask(i):
        w, r0, r1 = ((Wq, BQ0, BQ1), (Wk, BK0, BK1),
                     (Wv, BV0, BV1), (Wo, BO0, BO1))[i]
        s = _WSCR[i]
        np.add(np.ascontiguousarray(w, np.float32).view(np.uint32),
               0x8000, out=s)
        np.right_shift(s, 16, out=s)
        if i < 3:
            blob[:, r0:r1, :] = s.reshape(D, 8, P).transpose(1, 0, 2)[_PERMW]
        else:
            blob[:, r0:r1, :] = s.reshape(8, P, D)[_PERMW].reshape(
                8, r1 - r0, P)

    pool = _get_pool()
    fs = [pool.submit(xtask, c) for c in range(8)]
    fs += [pool.submit(wtask, i) for i in range(4)]
    for f in fs:
        f.result()
    return {"blob": blob.reshape(8 * NBLOB, P).view(ml_dtypes.bfloat16)}


def _finalize(out_bf_flat, bo):
    # fresh output each call (callers may hold references across calls);
    # widen bf16 bits to f32 in place per row-chunk on the thread pool
    u16 = np.asarray(out_bf_flat).view(np.uint16).reshape(B * S, D)
    out = np.empty((B * S, D), np.float32)
    ov = out.view(np.uint32)
    bo32 = np.asarray(bo, np.float32)
    addb = bool(bo32.any())

    def task(c):
        dst = ov[c * XSH:(c + 1) * XSH]
        dst[:] = u16[c * XSH:(c + 1) * XSH]
        np.left_shift(dst, 16, out=dst)
        if addb:
            f = out[c * XSH:(c + 1) * XSH]
            np.add(f, bo32[None, :], out=f)

    pool = _get_pool()
    for f in [pool.submit(task, c) for c in range(8)]:
        f.result()
    return out.reshape(B, S, D)


def _in_maps(gl):
    return [{"blob": gl["blob"][c * NBLOB:(c + 1) * NBLOB]} for c in range(8)]


def _get_cached_runner():
    """jit-compiled shard_map over the 8 cores, built once and reused.

    Mirrors bass2jax.run_bass_via_pjrt exactly (same primitive, same NEFF,
    same donation scheme) but keeps the jitted callable alive so warm calls
    skip re-tracing/re-lowering."""
    global _CACHED
    if _CACHED is not None:
        return _CACHED
    import jax
    from jax.sharding import Mesh, PartitionSpec
    from jax.experimental.shard_map import shard_map
    from concourse import mybir
    from concourse.bass2jax import (
        _bass_exec_p, install_neuronx_cc_hook, partition_id_tensor,
    )

    nc = _get_compiled()
    install_neuronx_cc_hook()
    partition_name = nc.partition_id_tensor.name if nc.partition_id_tensor else None
    in_names = []
    out_names = []
    out_avals = []
    out_shapes = []
    for alloc in nc.m.functions[0].allocations:
        if not isinstance(alloc, mybir.MemoryLocationSet):
            continue
        name = alloc.memorylocations[0].name
        if alloc.kind == "ExternalInput":
            if name != partition_name:
                in_names.append(name)
        elif alloc.kind == "ExternalOutput":
            shape = tuple(alloc.tensor_shape)
            dtype = mybir.dt.np(alloc.dtype)
            out_names.append(name)
            out_avals.append(jax.core.ShapedArray(shape, dtype))
            out_shapes.append((shape, dtype))
    n_params = len(in_names)
    # no donated zero buffers: the kernel writes every output element, so
    # outputs may start uninitialized and nothing extra goes over the wire
    in_names_all = list(in_names)
    if partition_name is not None:
        in_names_all.append(partition_name)

    def _body(*args):
        operands = list(args)
        if partition_name is not None:
            operands.append(partition_id_tensor())
        outs = _bass_exec_p.bind(
            *operands,
            out_avals=tuple(out_avals),
            in_names=tuple(in_names_all),
            out_names=tuple(out_names),
            lowering_input_output_aliases=(),
            sim_require_finite=True,
            sim_require_nnan=True,
            nc=nc,
        )
        return tuple(outs)

    devices = jax.devices()[:8]
    mesh = Mesh(np.asarray(devices), ("core",))
    # inputs are sharded per core; the output is replicated (the kernel
    # AllGathers it) so the host fetches it from one device only
    in_specs = (PartitionSpec("core"),) * n_params
    out_specs = (PartitionSpec(),) * len(out_names)
    sharded = jax.jit(
        shard_map(_body, mesh=mesh, in_specs=in_specs, out_specs=out_specs,
                  check_rep=False),
        keep_unused=True,
    )
    _CACHED = (sharded, in_names, out_names, out_shapes)
    return _CACHED


def run_spmd(x, Wq, Wk, Wv, Wo, bo, **spmd_kwargs):
    """Run the 8-core kernel; returns (full_output, BassKernelResults|None)."""
    global _FIRST_DONE
    gl = _globals_from_inputs(x, Wq, Wk, Wv, Wo)

    if spmd_kwargs or not _FIRST_DONE:
        # first (compile) call and trace/debug calls go through the stock
        # runner; warm calls reuse the jitted executable below
        from concourse.bass_utils import run_bass_kernel_spmd
        nc = _get_compiled()
        try:
            res = run_bass_kernel_spmd(nc, _in_maps(gl), list(range(8)),
                                       **spmd_kwargs)
        except Exception:
            if spmd_kwargs:
                raise
            # transient device wedge (NRT_EXEC_UNIT_UNRECOVERABLE etc.):
            # one retry
            res = run_bass_kernel_spmd(nc, _in_maps(gl), list(range(8)))
        _FIRST_DONE = True
        # warm the cached runner (trace/lower/XLA-compile) during the
        # cold call so the first timed warm call doesn't pay for it
        try:
            sharded, in_names, out_names, _ = _get_cached_runner()
            arrs = sharded(*[gl[name] for name in in_names])
            np.asarray(arrs[0])
        except Exception:
            pass
        # output is replicated across cores; take core 0's copy
        out_flat = res.results[0]["outp"]
        if not spmd_kwargs and _has_nan_bf16(out_flat):
            # transient device fault: rerun once
            res = run_bass_kernel_spmd(nc, _in_maps(gl), list(range(8)))
            out_flat = res.results[0]["outp"]
        out = _finalize(out_flat, bo)
        return out, res

    try:
        sharded, in_names, out_names, out_shapes = _get_cached_runner()
        out_arrs = sharded(*[gl[name] for name in in_names])
        out_flat = np.asarray(out_arrs[out_names.index("outp")])
        if _has_nan_bf16(out_flat):
            raise RuntimeError("nan in kernel output (transient fault)")
    except Exception:
        # recover from transient device failures via the stock runner
        from concourse.bass_utils import run_bass_kernel_spmd
        res = run_bass_kernel_spmd(_get_compiled(), _in_maps(gl),
                                   list(range(8)))
        out_flat = res.results[0]["outp"]
    return _finalize(out_flat, bo), None


def kernel(x, Wq, Wk, Wv, Wo, bo):
    out, _ = run_spmd(x, Wq, Wk, Wv, Wo, bo)
    return out


# revision 16
# speedup vs baseline: 1.1568x; 1.0433x over previous
import sys

if "/opt/trn_rl_repo" not in sys.path:
    sys.path.insert(0, "/opt/trn_rl_repo")

import numpy as np
import ml_dtypes

B, S, D, H = 2, 2048, 1024, 16
HPC = 4            # heads per core
HG = 256           # head-group width (HPC * DH)
DH = 64
P = 128
NS = S // P        # 16 s-tiles
ND = D // P        # 8 d-tiles
QC = 512           # q-chunk width
NQC = S // QC      # 4 chunks
NPAIR = 2          # head pairs per core
XSH = 512          # x rows shipped per core (B*S/8)

GX = [[0, 1, 2, 3], [4, 5, 6, 7]]           # batch groups
GW = [[0, 4], [1, 5], [2, 6], [3, 7]]       # weight-half pairs

_COMPILED = None
_CACHED = None
_FIRST_DONE = False


# blob row layout (all bf16, 128 cols): x shard rows then weight slices
BX0, BX1 = 0, 4096          # x shard   [512,1024]  -> [4096,128]
BQ0, BQ1 = 4096, 5120       # Wq half   [1024,128]
BK0, BK1 = 5120, 6144       # Wk half
BV0, BV1 = 6144, 7168       # Wv half
BO0, BO1 = 7168, 8192       # Wo half   [128,1024]  -> [1024,128]
NBLOB = 8192


def _emit(nc, tc, bass, mybir, make_identity, blob, outp):
    FR = mybir.dt.float32r
    F32 = mybir.dt.float32
    BF = mybir.dt.bfloat16
    Exp = mybir.ActivationFunctionType.Exp
    mult = mybir.AluOpType.mult
    add = mybir.AluOpType.add
    byp = mybir.AluOpType.bypass

    with (
        tc.tile_pool(name="persist", bufs=1) as pp,
        tc.tile_pool(name="psS", bufs=2, space="PSUM") as psa,
        tc.tile_pool(name="psPV", bufs=2, space="PSUM") as psb,
        tc.tile_pool(name="psO", bufs=2, space="PSUM") as psc,
        tc.tile_pool(name="wpool", bufs=1) as wp,
        tc.tile_pool(name="xcpool", bufs=2) as xcp,
        tc.tile_pool(name="xfpool", bufs=1) as xfp,
        tc.tile_pool(name="xtpool", bufs=2) as xtp,
        tc.tile_pool(name="eppool", bufs=2) as epp,
        tc.tile_pool(name="ctxpool", bufs=2) as cxp,
        tc.tile_pool(name="rpool", bufs=4) as rp,
        tc.tile_pool(name="bcpool", bufs=2) as bcp,
        tc.tile_pool(name="stagepool", bufs=2) as stp,
        tc.tile_pool(name="opool", bufs=2) as obp,
        tc.tile_pool(name="dram", bufs=1, space="DRAM") as dp,
    ):
        # ---- DRAM bounces + collective prologue ----
        # collectives can't touch I/O tensors: one bounce copy of the whole
        # input blob, then gather slices of it. Collectives treat buffers
        # linearly (.opt()), so the declared shapes of in/out only need to
        # agree byte-wise.
        bin_ = dp.tile([NBLOB, P], BF)
        xag = dp.tile([S, D], BF)          # full x[b] after group AllGather
        wqag = dp.tile([2, D, P], BF)      # [half, :, :]
        wkag = dp.tile([2, D, P], BF)
        wvag = dp.tile([2, D, P], BF)
        woag = dp.tile([2, P, D], BF)
        pout = dp.tile([S, D], BF)         # this core's out partial
        rsout = dp.tile([XSH, D], BF)      # reduced shard
        oag = dp.tile([B * S, D], BF)      # full output, replicated

        # bounce on the gpsimd queue: same engine as the collectives, so
        # the AllGathers are hardware-ordered after it
        nc.gpsimd.dma_start(bin_[:], blob[:])
        nc.gpsimd.collective_compute(
            "AllGather", byp, replica_groups=GX,
            ins=[bin_[BX0:BX1, :].opt()], outs=[xag[:].opt()],
        )
        nc.gpsimd.collective_compute(
            "AllGather", byp, replica_groups=GW,
            ins=[bin_[BQ0:BQ1, :].opt()], outs=[wqag[:].opt()],
        )
        nc.gpsimd.collective_compute(
            "AllGather", byp, replica_groups=GW,
            ins=[bin_[BK0:BK1, :].opt()], outs=[wkag[:].opt()],
        )
        nc.gpsimd.collective_compute(
            "AllGather", byp, replica_groups=GW,
            ins=[bin_[BV0:BV1, :].opt()], outs=[wvag[:].opt()],
        )
        nc.gpsimd.collective_compute(
            "AllGather", byp, replica_groups=GW,
            ins=[bin_[BO0:BO1, :].opt()], outs=[woag[:].opt()],
        )

        # persistent tensors
        qt = pp.tile([P, NPAIR, S], FR)        # Q^T pack: parts 0:64 head 2p, 64:128 head 2p+1
        kt = pp.tile([P, NPAIR, S], FR)        # K^T pack
        vv = pp.tile([P, NS, HPC, DH + 1], BF) # V natural per head + ones column
        ident = pp.tile([P, P], FR)
        tri = pp.tile([P, P], BF)              # 1.0 where part(k) <= free(q) else 0

        nc.vector.memset(vv[:, :, :, DH], 1.0)

        x_tiles = {}
        xt_tiles = {}
        ctx_tiles = {}

        def emit_xdma(cc):
            x_c = xcp.tile([P, 4, D], BF, name="x_c")
            for si in range(4):
                s = 4 * cc + si
                nc.gpsimd.dma_start(
                    out=x_c[:, si, :], in_=xag[s * P:(s + 1) * P, :])
            x_tiles[cc] = x_c

        # chunk 0 lands quarter-major in small pieces so the dt=0
        # transposes can start early; data is in flight while the masks
        # and identity build
        x_c = xcp.tile([P, 4, D], BF, name="x_c")
        engs0 = (nc.gpsimd, nc.scalar, nc.gpsimd, nc.scalar)
        q = D // 4
        for si in range(4):
            engs0[si].dma_start(out=x_c[:, si, 0:q],
                                in_=xag[si * P:(si + 1) * P, 0:q])
        x_tiles[0] = x_c
        # memset on float32r trips walrus ISA check; memset via f32 view
        nc.gpsimd.memset(ident[:].bitcast(F32), 0.0)
        make_identity(nc, ident[:], nomemset=True)
        for hh in range(1, 4):
            lo, hi = hh * q, (hh + 1) * q
            for si in range(4):
                engs0[si].dma_start(out=x_c[:, si, lo:hi],
                                    in_=xag[si * P:(si + 1) * P, lo:hi])
        nc.gpsimd.memset(tri[:], 0.0)
        # pred: -1 + p - f >= 0  (p > f) -> keep 0 ; else fill 1.0
        nc.gpsimd.affine_select(
            out=tri[:], in_=tri[:],
            compare_op=mybir.AluOpType.is_ge,
            fill=1.0, base=-1, channel_multiplier=1, pattern=[[-1, P]],
        )

        # weights: bf16 halves land in a staging tile, then one DVE copy
        # converts to the f32r layout the matmuls expect
        wq_sb = wp.tile([P, ND, HG], FR)
        wk_sb = wp.tile([P, ND, HG], FR)
        wv_sb = wp.tile([P, ND, HG], FR)
        wo_sb = wp.tile([P, NPAIR, D], FR)
        wstage = wp.tile([P, ND, HG], BF)
        wostage = wp.tile([P, NPAIR, D], BF)

        def load_w(wag, dst, eng):
            for dt in range(ND):
                for h in range(2):
                    eng.dma_start(
                        out=wstage[:, dt, h * P:(h + 1) * P],
                        in_=wag[h, dt * P:(dt + 1) * P, :],
                    )
            nc.vector.tensor_copy(dst[:], wstage[:])

        load_w(wqag, wq_sb, nc.sync)
        load_w(wkag, wk_sb, nc.sync)
        load_w(wvag, wv_sb, nc.scalar)
        # wo packed by head pair: partitions 0:64 head 2p, 64:128 head 2p+1;
        # pair pr rows = half pr of the gathered slice
        for pr in range(NPAIR):
            nc.sync.dma_start(
                out=wostage[0:DH, pr, :], in_=woag[pr, 0:DH, :])
            nc.sync.dma_start(
                out=wostage[DH:P, pr, :], in_=woag[pr, DH:P, :])
        nc.vector.tensor_copy(wo_sb[:], wostage[:])

        # phase-1 pieces use 1-bank tiles in the psO pool so their allocs
        # never wait on the slow exp drains that pace the psS pool
        def qk_pair(cc, pair):
            xT_c = xt_tiles[cc]
            ps_q = psc.tile([P, QC], F32, name="ps_o")
            for dt in range(ND):
                nc.tensor.matmul(
                    ps_q[:],
                    wq_sb[:, dt, pair * P:(pair + 1) * P],
                    xT_c[:, dt, :],
                    start=(dt == 0), stop=(dt == ND - 1),
                )
            nc.vector.tensor_copy(qt[:, pair, cc * QC:(cc + 1) * QC], ps_q[:])
            ps_k = psc.tile([P, QC], F32, name="ps_o")
            for dt in range(ND):
                nc.tensor.matmul(
                    ps_k[:],
                    wk_sb[:, dt, pair * P:(pair + 1) * P],
                    xT_c[:, dt, :],
                    start=(dt == 0), stop=(dt == ND - 1),
                )
            nc.vector.tensor_copy(kt[:, pair, cc * QC:(cc + 1) * QC], ps_k[:])

        def ph1_pieces(cc):
            def p_transpose():
                x_c = x_tiles.pop(cc)
                # bf16 wire -> f32 on ACT (idle during phase 1); per-si so
                # the dt=0 transposes don't wait on the whole chunk
                x_f = xfp.tile([P, 4, D], FR, name="x_f")
                for si in range(4):
                    nc.vector.tensor_copy(x_f[:, si, :], x_c[:, si, :])
                xT_c = xtp.tile([P, ND, QC], FR, name="xT_c")
                xt_tiles[cc] = xT_c
                for dt in range(ND):
                    ps_t = psc.tile([P, QC], FR, name="ps_o")
                    for si in range(4):
                        nc.tensor.transpose(
                            ps_t[:, si * P:(si + 1) * P],
                            x_f[:, si, dt * P:(dt + 1) * P],
                            ident[:],
                        )
                    nc.vector.tensor_copy(xT_c[:, dt, :], ps_t[:])

            def p_qk0():
                qk_pair(cc, 0)

            def p_qk1():
                qk_pair(cc, 1)

            def p_v():
                xT_c = xt_tiles.pop(cc)
                for si in range(4):
                    ps_v = psc.tile([P, QC], F32, name="ps_o")
                    for dt in range(ND):
                        nc.tensor.matmul(
                            ps_v[:, 0:HG],
                            xT_c[:, dt, si * P:(si + 1) * P],
                            wv_sb[:, dt, :],
                            start=(dt == 0), stop=(dt == ND - 1),
                        )
                    nc.vector.tensor_copy(
                        vv[:, 4 * cc + si, :, 0:DH], ps_v[:, 0:HG]
                    )

            return [p_transpose, p_qk0, p_qk1, p_v]

        def scores_unit_thunks(cc, h, ep):
            T = 4 * cc + 4
            pr = h // 2
            po = DH * (h % 2)
            thunks = []
            t = 0
            while t < T:
                if t + 2 <= 4 * cc:
                    # two full k-tiles share a 2-bank PSUM tile -> one exp
                    def u_pair(t=t):
                        ps_s = psa.tile([P, 2 * QC], F32, name="ps")
                        for uu in range(2):
                            nc.tensor.matmul(
                                ps_s[:, uu * QC:(uu + 1) * QC],
                                kt[po:po + DH, pr, (t + uu) * P:(t + uu + 1) * P],
                                qt[po:po + DH, pr, cc * QC:(cc + 1) * QC],
                                start=True, stop=True,
                            )
                        nc.scalar.activation(
                            ep[:, t * QC:(t + 2) * QC], ps_s[:], Exp, scale=0.125
                        )
                    thunks.append(u_pair)
                    t += 2
                else:
                    # diagonal k-tile: only causally-valid columns
                    jd = t - 4 * cc
                    lo = jd * P if jd > 0 else 0
                    def u_diag(t=t, lo=lo):
                        ps_s = psa.tile([P, 2 * QC], F32, name="ps")
                        nc.tensor.matmul(
                            ps_s[:, lo:QC],
                            kt[po:po + DH, pr, t * P:(t + 1) * P],
                            qt[po:po + DH, pr, cc * QC + lo:(cc + 1) * QC],
                            start=True, stop=True,
                        )
                        nc.scalar.activation(
                            ep[:, t * QC + lo:(t + 1) * QC], ps_s[:, lo:QC],
                            Exp, scale=0.125,
                        )
                    thunks.append(u_diag)
                    t += 1
            return thunks

        def tri_fixups(cc, ep):
            # causal fixups on the 4 diagonal k-tiles (cols < jd*P are
            # never read: PV matmuls are col-trimmed the same way)
            for jd in range(4):
                t2 = 4 * cc + jd
                base = t2 * QC + jd * P
                nc.vector.tensor_tensor(
                    ep[:, base:base + P], ep[:, base:base + P], tri[:], op=mult
                )

        def pv_thunks(cc, h, ep, ps_ctx):
            T = 4 * cc + 4
            thunks = []
            for t in range(T):
                jd = t - 4 * cc
                lo = jd * P if jd > 0 else 0
                def u(t=t, lo=lo):
                    nc.tensor.matmul(
                        ps_ctx[:, lo:QC],
                        vv[:, t, h, :],
                        ep[:, t * QC + lo:(t + 1) * QC],
                        start=(t == 0), stop=(t == T - 1),
                    )
                thunks.append(u)
            return thunks

        def emit_pv_finish(cc, h, ps_ctx, recip):
            ctx_c = ctx_tiles[cc]
            # broadcast recip across 64 partitions on the Pool engine
            # (SBUF->SBUF; tensor_tensor may read only one PSUM input)
            bc_sb = bcp.tile([DH, QC], F32, name="bc_sb")
            nc.gpsimd.partition_broadcast(bc_sb[:], recip[:])
            pr, odd = divmod(h, 2)
            if odd == 0:
                nc.vector.tensor_tensor(
                    ctx_c[0:DH, pr, :], ps_ctx[0:DH, :], bc_sb[:], op=mult
                )
            else:
                # odd head lands on partitions 64:128 via SBUF->SBUF DMA
                stage = stp.tile([DH, QC], FR, name="stage")
                nc.vector.tensor_tensor(
                    stage[:], ps_ctx[0:DH, :], bc_sb[:], op=mult
                )
                nc.gpsimd.dma_start(out=ctx_c[DH:P, pr, :], in_=stage[:])

        def emit_outproj(cc, last=False):
            ctx_c = ctx_tiles.pop(cc)
            # all partial stores go on the gpsimd queue: the ReduceScatter
            # that consumes pout runs there too, so it is hardware-ordered
            # after every store (device exec is not the wall-clock
            # bottleneck; transfer is)
            for jq in range(4):
                i = 4 * cc + jq
                out_sb = obp.tile([P, D], BF)
                for nk in range(2):
                    ps_o = psc.tile([P, QC], F32)
                    for pr in range(NPAIR):
                        nc.tensor.matmul(
                            ps_o[:],
                            ctx_c[:, pr, jq * P:(jq + 1) * P],
                            wo_sb[:, pr, nk * QC:(nk + 1) * QC],
                            start=(pr == 0), stop=(pr == NPAIR - 1),
                        )
                    nc.vector.tensor_copy(out_sb[:, nk * QC:(nk + 1) * QC], ps_o[:])
                    nc.gpsimd.dma_start(
                        out=pout[i * P:(i + 1) * P,
                                 nk * QC:(nk + 1) * QC],
                        in_=out_sb[:, nk * QC:(nk + 1) * QC],
                    )

        # ---- driver: chunk-interleaved software pipeline ----
        # Per head-block: scores(h) psa units are ACT-paced; PV(h-1)
        # chain matmuls are interleaved between them so the PE FIFO
        # always has runnable work while an exp drains a psa buffer.
        prev = [None]
        nfin = {0: 0, 1: 0, 2: 0, 3: 0}
        # last finish of each chunk is an even head: no Pool shift on
        # the critical tail before outproj
        HEAD_ORDER = (1, 0, 3, 2)

        def head_block(cc, h, piece):
            if cc not in ctx_tiles:
                ctx_tiles[cc] = cxp.tile([P, NPAIR, QC], FR, name="ctx_c")
            ep = epp.tile([P, NS * QC], BF, name="ep")
            su = scores_unit_thunks(cc, h, ep)
            pvt, fin = [], None
            if prev[0] is not None:
                pcc, ph2, pep = prev[0]
                ps_ctx = psb.tile([DH + 1, QC], F32, name="pv")
                pvt = pv_thunks(pcc, ph2, pep, ps_ctx)
                fin = (pcc, ph2, ps_ctx)
            su[0]()
            if len(su) > 1:
                su[1]()
            rest = su[2:]
            nslots = len(rest) + 1
            done = 0
            for j in range(nslots):
                want = ((j + 1) * len(pvt)) // nslots
                while done < want:
                    pvt[done]()
                    done += 1
                if j < len(rest):
                    rest[j]()
            # pv_finish goes on the DVE queue ahead of the fixups so the
            # psb slot frees before the block-end DVE burst
            ofin = None
            if fin is not None:
                recip = rp.tile([1, QC], F32)
                nc.vector.reciprocal(recip[:], fin[2][DH:DH + 1, :])
                pcc, ph2, ps_ctx = fin
                emit_pv_finish(pcc, ph2, ps_ctx, recip)
                nfin[pcc] += 1
                if nfin[pcc] == HPC:
                    ofin = pcc
            tri_fixups(cc, ep)
            if piece is not None:
                piece()
            if ofin is not None:
                emit_outproj(ofin)
            prev[0] = (cc, h, ep)

        def attn(cc, pieces=()):
            it = iter(pieces)
            for h in HEAD_ORDER:
                head_block(cc, h, next(it, None))

        emit_xdma(1)
        for p in ph1_pieces(0):
            p()
        emit_xdma(2)
        for p in ph1_pieces(1):
            p()
        emit_xdma(3)
        attn(0, ph1_pieces(2))
        attn(1, ph1_pieces(3))
        attn(3)
        attn(2)
        # flush the last head
        pcc, ph2, pep = prev[0]
        ps_ctx = psb.tile([DH + 1, QC], F32, name="pv")
        for u in pv_thunks(pcc, ph2, pep, ps_ctx):
            u()
        recip = rp.tile([1, QC], F32)
        nc.vector.reciprocal(recip[:], ps_ctx[DH:DH + 1, :])
        emit_pv_finish(pcc, ph2, ps_ctx, recip)
        emit_outproj(pcc, last=True)

        # ---- epilogue: sum the 4 head-group partials on device, then
        # gather the full output on every core so the host fetches it
        # from a single device ----
        nc.gpsimd.collective_compute(
            "ReduceScatter", add, replica_groups=GX,
            ins=[pout[:].opt()], outs=[rsout[:].opt()],
        )
        nc.gpsimd.collective_compute(
            "AllGather", byp, replica_groups=[list(range(8))],
            ins=[rsout[:].opt()], outs=[oag[:].opt()],
        )
        nc.gpsimd.dma_start(out=outp[:], in_=oag[:])


def _build():
    import concourse.bass as bass
    import concourse.tile as tile
    from concourse import bacc, mybir
    from concourse.masks import make_identity

    BF = mybir.dt.bfloat16

    nc = bacc.Bacc(
        "TRN2", target_bir_lowering=False, debug=False,
        enable_asserts=True, num_devices=8,
    )
    blob = nc.dram_tensor("blob", [NBLOB, P], BF, kind="ExternalInput")
    outp = nc.dram_tensor("outp", [B * S, D], BF, kind="ExternalOutput")

    with tile.TileContext(nc) as tc:
        _emit(nc, tc, bass, mybir, make_identity, blob, outp)
    nc.compile()
    return nc


def _get_compiled():
    global _COMPILED
    if _COMPILED is None:
        _COMPILED = _build()
    return _COMPILED


def _to_u16(a):
    """f32 -> bf16 bits (round-to-nearest, ties away) as uint16."""
    u = np.ascontiguousarray(a, dtype=np.float32).view(np.uint32)
    return ((u + 0x8000) >> 16).astype(np.uint16)


def _from_bf16_f32(a):
    """bf16 (or uint16 view) -> exact f32."""
    u = np.asarray(a).view(np.uint16)
    return (u.astype(np.uint32) << 16).view(np.float32)


def _has_nan_bf16(a):
    """True if any bf16 element is nan/inf (transient device fault sign)."""
    u = np.asarray(a).view(np.uint16)
    return bool(((u & 0x7FFF) >= 0x7F80).any())


# core c (b = c//4, g = c%4) ships half b of head-group g's slice; the
# slice at column offset g*HG + b*P is 128-column block j = 2g + b
_PERMW = np.array([(c % 4) * 2 + (c // 4) for c in range(8)])
_BLOB_SCRATCH = None
_XSCR = None
_WSCR = None
_POOL = None


def _get_pool():
    global _POOL
    if _POOL is None:
        from concurrent.futures import ThreadPoolExecutor
        _POOL = ThreadPoolExecutor(max_workers=8)
    return _POOL


def _globals_from_inputs(x, Wq, Wk, Wv, Wo):
    """One packed (8*NBLOB, 128) bf16 blob, core-major, matching the
    shard_map layout run_bass_via_pjrt uses. Conversion runs on a thread
    pool with preallocated scratch (numpy ufuncs release the GIL)."""
    global _BLOB_SCRATCH, _XSCR, _WSCR
    if _BLOB_SCRATCH is None:
        _BLOB_SCRATCH = np.empty((8, NBLOB, P), np.uint16)
        _XSCR = [np.empty((XSH, D), np.uint32) for _ in range(8)]
        _WSCR = [np.empty((D, D), np.uint32) for _ in range(4)]
    blob = _BLOB_SCRATCH
    xf = np.ascontiguousarray(x, np.float32).reshape(B * S, D)

    def xtask(c):
        s = _XSCR[c]
        np.add(xf[c * XSH:(c + 1) * XSH].view(np.uint32), 0x8000, out=s)
        np.right_shift(s, 16, out=s)
        blob[c, BX0:BX1, :].reshape(XSH, D)[:] = s

    def wtask(i):
        w, r0, r1 = ((Wq, BQ0, BQ1), (Wk, BK0, BK1),
                     (Wv, BV0, BV1), (Wo, BO0, BO1))[i]
        s = _WSCR[i]
        np.add(np.ascontiguousarray(w, np.float32).view(np.uint32),
               0x8000, out=s)
        np.right_shift(s, 16, out=s)
        if i < 3:
            blob[:, r0:r1, :] = s.reshape(D, 8, P).transpose(1, 0, 2)[_PERMW]
        else:
            blob[:, r0:r1, :] = s.reshape(8, P, D)[_PERMW].reshape(
                8, r1 - r0, P)

    pool = _get_pool()
    fs = [pool.submit(xtask, c) for c in range(8)]
    fs += [pool.submit(wtask, i) for i in range(4)]
    for f in fs:
        f.result()
    return {"blob": blob.reshape(8 * NBLOB, P).view(ml_dtypes.bfloat16)}


_WU16 = None


def _pack_upload(x, Wq, Wk, Wv, Wo, mesh, devices):
    """Pack per-core blob shards and start each shard's upload as soon as
    it is ready, overlapping host conversion with the wire. Returns a
    global jax Array already sharded the way the jit expects."""
    global _BLOB_SCRATCH, _XSCR, _WSCR, _WU16
    import concurrent.futures as cf
    import jax
    from jax.sharding import NamedSharding, PartitionSpec
    if _BLOB_SCRATCH is None:
        _BLOB_SCRATCH = np.empty((8, NBLOB, P), np.uint16)
        _XSCR = [np.empty((XSH, D), np.uint32) for _ in range(8)]
        _WSCR = [np.empty((D, D), np.uint32) for _ in range(4)]
    if _WU16 is None:
        _WU16 = [np.empty((D, D), np.uint16) for _ in range(4)]
    pool = _get_pool()
    xf = np.ascontiguousarray(x, np.float32).reshape(B * S, D)
    wf = [np.ascontiguousarray(w, np.float32) for w in (Wq, Wk, Wv, Wo)]

    def wconv(i):
        s = _WSCR[i]
        np.add(wf[i].view(np.uint32), 0x8000, out=s)
        np.right_shift(s, 16, out=s)
        _WU16[i][:] = s

    for f in [pool.submit(wconv, i) for i in range(4)]:
        f.result()

    def ctask(c):
        s = _XSCR[c]
        np.add(xf[c * XSH:(c + 1) * XSH].view(np.uint32), 0x8000, out=s)
        np.right_shift(s, 16, out=s)
        bc = _BLOB_SCRATCH[c]
        bc[BX0:BX1, :].reshape(XSH, D)[:] = s
        o = (c % 4) * HG + (c // 4) * P
        bc[BQ0:BQ1, :] = _WU16[0][:, o:o + P]
        bc[BK0:BK1, :] = _WU16[1][:, o:o + P]
        bc[BV0:BV1, :] = _WU16[2][:, o:o + P]
        bc[BO0:BO1, :] = _WU16[3][o:o + P, :].reshape(BO1 - BO0, P)
        return c

    darrs = [None] * 8
    for f in cf.as_completed([pool.submit(ctask, c) for c in range(8)]):
        c = f.result()
        darrs[c] = jax.device_put(
            _BLOB_SCRATCH[c].view(ml_dtypes.bfloat16), devices[c])
    return jax.make_array_from_single_device_arrays(
        (8 * NBLOB, P), NamedSharding(mesh, PartitionSpec("core")), darrs)


def _finalize(out_bf_flat, bo):
    # fresh output each call (callers may hold references across calls);
    # widen bf16 bits to f32 in place per row-chunk on the thread pool
    u16 = np.asarray(out_bf_flat).view(np.uint16).reshape(B * S, D)
    out = np.empty((B * S, D), np.float32)
    ov = out.view(np.uint32)
    bo32 = np.asarray(bo, np.float32)
    addb = bool(bo32.any())

    def task(c):
        dst = ov[c * XSH:(c + 1) * XSH]
        dst[:] = u16[c * XSH:(c + 1) * XSH]
        np.left_shift(dst, 16, out=dst)
        if addb:
            f = out[c * XSH:(c + 1) * XSH]
            np.add(f, bo32[None, :], out=f)

    pool = _get_pool()
    for f in [pool.submit(task, c) for c in range(8)]:
        f.result()
    return out.reshape(B, S, D)


def _in_maps(gl):
    return [{"blob": gl["blob"][c * NBLOB:(c + 1) * NBLOB]} for c in range(8)]


def _get_cached_runner():
    """jit-compiled shard_map over the 8 cores, built once and reused.

    Mirrors bass2jax.run_bass_via_pjrt exactly (same primitive, same NEFF,
    same donation scheme) but keeps the jitted callable alive so warm calls
    skip re-tracing/re-lowering."""
    global _CACHED
    if _CACHED is not None:
        return _CACHED
    import jax
    from jax.sharding import Mesh, PartitionSpec
    from jax.experimental.shard_map import shard_map
    from concourse import mybir
    from concourse.bass2jax import (
        _bass_exec_p, install_neuronx_cc_hook, partition_id_tensor,
    )

    nc = _get_compiled()
    install_neuronx_cc_hook()
    partition_name = nc.partition_id_tensor.name if nc.partition_id_tensor else None
    in_names = []
    out_names = []
    out_avals = []
    out_shapes = []
    for alloc in nc.m.functions[0].allocations:
        if not isinstance(alloc, mybir.MemoryLocationSet):
            continue
        name = alloc.memorylocations[0].name
        if alloc.kind == "ExternalInput":
            if name != partition_name:
                in_names.append(name)
        elif alloc.kind == "ExternalOutput":
            shape = tuple(alloc.tensor_shape)
            dtype = mybir.dt.np(alloc.dtype)
            out_names.append(name)
            out_avals.append(jax.core.ShapedArray(shape, dtype))
            out_shapes.append((shape, dtype))
    n_params = len(in_names)
    # no donated zero buffers: the kernel writes every output element, so
    # outputs may start uninitialized and nothing extra goes over the wire
    in_names_all = list(in_names)
    if partition_name is not None:
        in_names_all.append(partition_name)

    def _body(*args):
        operands = list(args)
        if partition_name is not None:
            operands.append(partition_id_tensor())
        outs = _bass_exec_p.bind(
            *operands,
            out_avals=tuple(out_avals),
            in_names=tuple(in_names_all),
            out_names=tuple(out_names),
            lowering_input_output_aliases=(),
            sim_require_finite=True,
            sim_require_nnan=True,
            nc=nc,
        )
        return tuple(outs)

    devices = jax.devices()[:8]
    mesh = Mesh(np.asarray(devices), ("core",))
    # inputs are sharded per core; the output is replicated (the kernel
    # AllGathers it) so the host fetches it from one device only
    in_specs = (PartitionSpec("core"),) * n_params
    out_specs = (PartitionSpec(),) * len(out_names)
    sharded = jax.jit(
        shard_map(_body, mesh=mesh, in_specs=in_specs, out_specs=out_specs,
                  check_rep=False),
        keep_unused=True,
    )
    _CACHED = (sharded, in_names, out_names, out_shapes, mesh, devices)
    return _CACHED


def run_spmd(x, Wq, Wk, Wv, Wo, bo, **spmd_kwargs):
    """Run the 8-core kernel; returns (full_output, BassKernelResults|None)."""
    global _FIRST_DONE

    if spmd_kwargs or not _FIRST_DONE:
        # first (compile) call and trace/debug calls go through the stock
        # runner; warm calls reuse the jitted executable below
        from concourse.bass_utils import run_bass_kernel_spmd
        nc = _get_compiled()
        gl = _globals_from_inputs(x, Wq, Wk, Wv, Wo)
        try:
            res = run_bass_kernel_spmd(nc, _in_maps(gl), list(range(8)),
                                       **spmd_kwargs)
        except Exception:
            if spmd_kwargs:
                raise
            # transient device wedge (NRT_EXEC_UNIT_UNRECOVERABLE etc.):
            # one retry
            res = run_bass_kernel_spmd(nc, _in_maps(gl), list(range(8)))
        _FIRST_DONE = True
        # warm the cached runner (trace/lower/XLA-compile) during the
        # cold call so the first timed warm call doesn't pay for it
        try:
            sharded, in_names, out_names, _, mesh, devices = \
                _get_cached_runner()
            arrs = sharded(*[gl[name] for name in in_names])
            np.asarray(arrs[0])
        except Exception:
            pass
        # output is replicated across cores; take core 0's copy
        out_flat = res.results[0]["outp"]
        if not spmd_kwargs and _has_nan_bf16(out_flat):
            # transient device fault: rerun once
            res = run_bass_kernel_spmd(nc, _in_maps(gl), list(range(8)))
            out_flat = res.results[0]["outp"]
        out = _finalize(out_flat, bo)
        return out, res

    # note: starting per-shard device_puts as each pack task finishes was
    # measured SLOWER here (~+40ms: 8 separate transfer RPCs cost more
    # than the jit's batched staging), so the packed blob goes through
    # the jit call directly
    gl = _globals_from_inputs(x, Wq, Wk, Wv, Wo)
    try:
        sharded, in_names, out_names, out_shapes, mesh, devices = \
            _get_cached_runner()
        out_arrs = sharded(*[gl[name] for name in in_names])
        out_flat = np.asarray(out_arrs[out_names.index("outp")])
        if _has_nan_bf16(out_flat):
            raise RuntimeError("nan in kernel output (transient fault)")
    except Exception:
        # recover from transient device failures via the stock runner
        from concourse.bass_utils import run_bass_kernel_spmd
        res = run_bass_kernel_spmd(_get_compiled(), _in_maps(gl),
                                   list(range(8)))
        out_flat = res.results[0]["outp"]
    return _finalize(out_flat, bo), None


def kernel(x, Wq, Wk, Wv, Wo, bo):
    out, _ = run_spmd(x, Wq, Wk, Wv, Wo, bo)
    return out
